# revision 1
# baseline (speedup 1.0000x reference)
"""LSTM warmup + autoregressive decode kernel for Trainium2 (Bass/Tile).

Reference computation (per batch row):
  h,c = 0
  for t in range(T):  h,c = LSTMstep(x_t)        # warmup over input seq
  pred0 = h @ Wd + bd
  for d in range(out_steps-1): h,c = LSTMstep(pred_d); pred_{d+1} = h@Wd+bd
  out[b, s, f] = pred_s

Strategy: data-parallel over 8 NeuronCores (B=4096 -> 512/core); the
sequential time loop stays local per shard.  On-chip everything is kept in a
*transposed* layout (partitions = unit/feature index, free dim = batch):
z^T[1024, B] per step via fp32r matmuls (weights stationary, x^T/h^T moving),
gates as [128, 2*B] tiles, so h^T feeds the next step's matmuls directly and
the recurrence needs no transposes.  PE transposes (via identity) only stage
x^T from the input layout and emit the output layout.

The autoregressive decode is algebraically fused: since
  z_{t+1} = pred_t @ W + h_t @ U + b   and   pred_t = h_t @ Wd + bd,
we precompute Ud = U + Wd@W and bias b + bd@W on the host, making each decode
step a single K=256 recurrence with no pred -> x round trip on the critical
path (pred is still computed, but only as output staging).
"""

import sys

for _p in ("/opt/trn_rl_repo", "/root/.axon_site/_ro/trn_rl_repo"):
    if _p not in sys.path:
        sys.path.insert(0, _p)

import numpy as np

import concourse.bacc as bacc
import concourse.mybir as mybir
import concourse.tile as tile
from concourse import bass_utils

F32 = mybir.dt.float32
F32R = mybir.dt.float32r
AF = mybir.ActivationFunctionType

N_CORES = 8
F = 64          # input/output feature dim
U = 256         # lstm units
U4 = 4 * U      # gate rows
# gate order in the 1024-row z layout (keras order i,f,g,o)
G_I, G_F, G_G, G_O = 0, 1, 2, 3


def build_program(B, T, out_steps, use_f32r=True):
    """Build the single-core SPMD program for a batch shard of size B."""
    assert B % 128 == 0
    NB = B // 128
    assert T % 2 == 0
    n_in_pairs = T // 2

    nc = bacc.Bacc("TRN2", target_bir_lowering=False, debug=False, num_devices=1)

    WDT = F32R if use_f32r else F32
    xin = nc.dram_tensor("xin", [B, T, F], F32, kind="ExternalInput").ap()
    w2d = nc.dram_tensor("w2", [128, U4], WDT, kind="ExternalInput").ap()
    u2d = nc.dram_tensor("u2", [128, 2 * U4], WDT, kind="ExternalInput").ap()
    ud2d = nc.dram_tensor("ud2", [128, 2 * U4], WDT, kind="ExternalInput").ap()
    wdd_d = nc.dram_tensor("wdd", [128, 2 * F], WDT, kind="ExternalInput").ap()
    ident_d = nc.dram_tensor("ident", [128, 128], F32, kind="ExternalInput").ap()
    bias8_d = nc.dram_tensor("bias8", [128, 8], F32, kind="ExternalInput").ap()
    bias8d_d = nc.dram_tensor("bias8d", [128, 8], F32, kind="ExternalInput").ap()
    bdup_d = nc.dram_tensor("bdup", [128, 1], F32, kind="ExternalInput").ap()
    yout = nc.dram_tensor("yout", [B, out_steps, F], F32, kind="ExternalOutput").ap()

    xin_f = xin.rearrange("b t f -> b (t f)")
    yout_f = yout.rearrange("b s f -> b (s f)")

    def mmt(ap):
        return ap.bitcast(F32R) if use_f32r else ap

    rnd = mmt  # producers feeding fp32r matmuls must declare fp32r outputs

    with tile.TileContext(nc) as tc:
        import contextlib

        with contextlib.ExitStack() as ctx:
            wpool = ctx.enter_context(tc.tile_pool(name="wpool", bufs=1))
            dpool = ctx.enter_context(tc.tile_pool(name="dpool", bufs=8))
            xpool = ctx.enter_context(tc.tile_pool(name="xpool", bufs=6))
            gpool = ctx.enter_context(tc.tile_pool(name="gpool", bufs=2))
            opool = ctx.enter_context(tc.tile_pool(name="opool", bufs=3))
            prpool = ctx.enter_context(tc.tile_pool(name="prpool", bufs=4))
            zpool = ctx.enter_context(tc.tile_pool(name="zpool", bufs=6, space="PSUM"))
            upool = ctx.enter_context(tc.tile_pool(name="upool", bufs=2, space="PSUM"))

            # ---- constants / weights ----
            w2 = wpool.tile([128, U4], WDT)        # W duplicated rows 0:64 / 64:128
            nc.sync.dma_start(w2[:], w2d[:])
            u2 = wpool.tile([128, 2 * U4], WDT)    # warmup U, k-chunks side by side
            nc.sync.dma_start(u2[:], u2d[:])
            ud2 = wpool.tile([128, 2 * U4], WDT)   # decode U + Wd@W
            nc.sync.dma_start(ud2[:], ud2d[:])
            wdd = wpool.tile([128, 2 * F], WDT)    # Wd k-chunks side by side
            nc.sync.dma_start(wdd[:], wdd_d[:])
            ident = wpool.tile([128, 128], F32)
            nc.sync.dma_start(ident[:], ident_d[:])
            bias8 = wpool.tile([128, 8], F32)
            nc.sync.dma_start(bias8[:], bias8_d[:])
            bias8d = wpool.tile([128, 8], F32)
            nc.sync.dma_start(bias8d[:], bias8d_d[:])
            bdup = wpool.tile([128, 1], F32)
            nc.sync.dma_start(bdup[:], bdup_d[:])

            xpairs = {}   # pair idx -> SBUF [128, B] x^T for steps 2p, 2p+1
            preds = {}    # decode pred idx -> SBUF [64, B] pred^T

            def emit_in_pair(p):
                xp = upool.tile([128, B], F32, tag="util", name=f"xtp{p}")
                for bc in range(NB):
                    dt_in = dpool.tile([128, 128], F32, tag="din", name=f"din{p}_{bc}")
                    nc.sync.dma_start(
                        dt_in[:],
                        xin_f[128 * bc : 128 * (bc + 1), 128 * p : 128 * (p + 1)],
                    )
                    nc.tensor.transpose(
                        xp[:, 128 * bc : 128 * (bc + 1)], dt_in[:], ident[:]
                    )
                xs = xpool.tile([128, B], F32, tag="xpair", name=f"xpair{p}")
                nc.vector.tensor_copy(rnd(xs[:]), xp[:])
                xpairs[p] = xs

            GATES = ((G_I, AF.Sigmoid, "gi"), (G_G, AF.Tanh, "gg"),
                     (G_F, AF.Sigmoid, "gf"), (G_O, AF.Sigmoid, "go"))

            def lstm_step(t, h_prev, c_prev, x_src=None, rb=0):
                """One LSTM step.  decode (x_src None): fused Ud recurrence.
                Returns (h, c) tiles [128, 2*B] in (uchunk, batch) layout."""
                uw = u2 if x_src is not None else ud2
                bias = bias8 if x_src is not None else bias8d
                zt = {}
                for ch in (0, 1):
                    for q, _, _ in GATES:
                        zq = zpool.tile([128, B], F32, tag="z", name=f"z{t}_{q}_{ch}")
                        mcol = 256 * q + 128 * ch
                        first = True
                        if x_src is not None:
                            nc.tensor.matmul(
                                zq[:],
                                mmt(w2[rb : rb + 64, mcol : mcol + 128]),
                                mmt(x_src[rb : rb + 64, :]),
                                start=True,
                                stop=(h_prev is None),
                            )
                            first = False
                        if h_prev is not None:
                            nc.tensor.matmul(
                                zq[:],
                                mmt(uw[:, mcol : mcol + 128]),
                                mmt(h_prev[:, 0:B]),
                                start=first,
                                stop=False,
                            )
                            nc.tensor.matmul(
                                zq[:],
                                mmt(uw[:, U4 + mcol : U4 + mcol + 128]),
                                mmt(h_prev[:, B : 2 * B]),
                                start=False,
                                stop=True,
                            )
                        zt[(q, ch)] = zq

                g = {}
                for q, _, tg in GATES:
                    g[q] = gpool.tile([128, 2 * B], F32, tag=tg, name=f"g{t}_{q}")
                c_t = gpool.tile([128, 2 * B], F32, tag="c", name=f"c{t}")
                tc_t = gpool.tile([128, 2 * B], F32, tag="tc", name=f"tc{t}")
                h_t = gpool.tile([128, 2 * B], F32, tag="h", name=f"h{t}")
                m2 = gpool.tile([128, 2 * B], F32, tag="m2", name=f"m2_{t}")
                if c_prev is not None:
                    fc = gpool.tile([128, 2 * B], F32, tag="fc", name=f"fc{t}")

                for ch in (0, 1):
                    s = slice(B * ch, B * (ch + 1))
                    for q, func, _ in GATES:
                        bcol = 2 * q + ch
                        nc.scalar.activation(
                            g[q][:, s], zt[(q, ch)][:],
                            func, bias=bias[:, bcol : bcol + 1],
                        )
                        if q == G_G:
                            nc.vector.tensor_mul(m2[:, s], g[G_I][:, s], g[G_G][:, s])
                        elif q == G_F and c_prev is not None:
                            nc.vector.tensor_mul(fc[:, s], g[G_F][:, s], c_prev[:, s])
                            nc.vector.tensor_add(c_t[:, s], fc[:, s], m2[:, s])
                    if c_prev is None:
                        nc.vector.tensor_copy(c_t[:, s], m2[:, s])
                    nc.scalar.activation(tc_t[:, s], c_t[:, s], AF.Tanh)
                    nc.vector.tensor_mul(rnd(h_t[:, s]), g[G_O][:, s], tc_t[:, s])
                return h_t, c_t

            def emit_pred(d, h_t):
                """pred_d^T = Wd^T h + bd -> [64, B] SBUF tile."""
                pp = upool.tile([64, B], F32, tag="util", name=f"predp{d}")
                nc.tensor.matmul(
                    pp[:], mmt(wdd[:, 0:F]), mmt(h_t[:, 0:B]), start=True, stop=False
                )
                nc.tensor.matmul(
                    pp[:], mmt(wdd[:, F : 2 * F]), mmt(h_t[:, B : 2 * B]),
                    start=False, stop=True,
                )
                ps = prpool.tile([64, B], F32, tag="prp", name=f"prsb{d}")
                nc.scalar.activation(ps[:], pp[:], AF.Identity, bias=bdup[0:64, 0:1])
                preds[d] = ps

            def emit_out_step(d):
                """Transpose pred_d to [batch, feat] layout and DMA out."""
                ps = preds.pop(d)
                tp = upool.tile([128, NB * F], F32, tag="util", name=f"otp{d}")
                for bc in range(NB):
                    nc.tensor.matmul(
                        tp[:, F * bc : F * (bc + 1)],
                        ps[:, 128 * bc : 128 * (bc + 1)],
                        ident[0:64, 0:F],
                        is_transpose=True,
                    )
                osb = opool.tile([128, NB * F], F32, tag="ot", name=f"osb{d}")
                nc.vector.tensor_copy(osb[:], tp[:])
                for bc in range(NB):
                    nc.sync.dma_start(
                        yout_f[128 * bc : 128 * (bc + 1), F * d : F * (d + 1)],
                        osb[:, F * bc : F * bc + F],
                    )

            # ---- warmup over the input sequence ----
            emit_in_pair(0)
            if n_in_pairs > 1:
                emit_in_pair(1)
            h_t = c_t = None
            for t in range(T):
                p, rb = t // 2, 64 * (t % 2)
                if t % 2 == 0 and p + 2 < n_in_pairs:
                    emit_in_pair(p + 2)
                h_t, c_t = lstm_step(t, h_t, c_t, x_src=xpairs[p], rb=rb)
                if t % 2 == 1:
                    del xpairs[p]

            # ---- autoregressive decode (fused recurrence) ----
            # pred_k/output emission lags one step so the recurrence matmuls
            # keep scheduling priority.
            hs = {0: h_t}
            for k in range(1, out_steps):
                h_t, c_t = lstm_step(T + k, h_t, c_t)
                hs[k] = h_t
                emit_pred(k - 1, hs.pop(k - 1))
                if k >= 2:
                    emit_out_step(k - 2)
            emit_pred(out_steps - 1, hs.pop(out_steps - 1))
            if out_steps >= 2:
                emit_out_step(out_steps - 2)
            emit_out_step(out_steps - 1)

    nc.compile()
    return nc


_CACHE = {}


def _get_program(key):
    if key not in _CACHE:
        _CACHE[key] = build_program(*key)
    return _CACHE[key]


def _host_prep(W, Uk, b, Wd, bd):
    W64 = W.astype(np.float64)
    Ud = (Uk.astype(np.float64) + Wd.astype(np.float64) @ W64).astype(np.float32)
    bdec = (b.astype(np.float64) + bd.astype(np.float64) @ W64).astype(np.float32)
    w2 = np.concatenate([W, W], axis=0).astype(np.float32)            # [128, 1024]
    u2 = np.concatenate([Uk[0:128], Uk[128:256]], axis=1).astype(np.float32)
    ud2 = np.concatenate([Ud[0:128], Ud[128:256]], axis=1).astype(np.float32)
    wdd = np.concatenate([Wd[0:128], Wd[128:256]], axis=1).astype(np.float32)
    ident = np.eye(128, dtype=np.float32)
    bias8 = np.ascontiguousarray(b.reshape(8, 128).T.astype(np.float32))
    bias8d = np.ascontiguousarray(bdec.reshape(8, 128).T.astype(np.float32))
    bdup = np.concatenate([bd, bd]).reshape(128, 1).astype(np.float32)
    return {
        "w2": w2, "u2": u2, "ud2": ud2, "wdd": wdd, "ident": ident,
        "bias8": bias8, "bias8d": bias8d, "bdup": bdup,
    }


def kernel(inputs, W, U, b, Wd, bd, out_steps):
    inputs = np.asarray(inputs, dtype=np.float32)
    W = np.asarray(W, dtype=np.float32)
    U_ = np.asarray(U, dtype=np.float32)
    b_ = np.asarray(b, dtype=np.float32)
    Wd = np.asarray(Wd, dtype=np.float32)
    bd = np.asarray(bd, dtype=np.float32)
    out_steps = int(out_steps)

    B_full, T, _ = inputs.shape
    assert B_full % N_CORES == 0
    Bc = B_full // N_CORES

    nc = _get_program((Bc, T, out_steps, True))
    shared = _host_prep(W, U_, b_, Wd, bd)
    in_maps = [
        {"xin": np.ascontiguousarray(inputs[i * Bc : (i + 1) * Bc]), **shared}
        for i in range(N_CORES)
    ]
    res = bass_utils.run_bass_kernel_spmd(nc, in_maps, core_ids=list(range(N_CORES)))
    out = np.concatenate([res.results[i]["yout"] for i in range(N_CORES)], axis=0)
    return out



# revision 16
# speedup vs baseline: 1.2855x; 1.2855x over previous
"""LSTM warmup + autoregressive decode kernel for Trainium2 (Bass/Tile).

Reference computation (per batch row):
  h,c = 0
  for t in range(T):  h,c = LSTMstep(x_t)        # warmup over input seq
  pred0 = h @ Wd + bd
  for d in range(out_steps-1): h,c = LSTMstep(pred_d); pred_{d+1} = h@Wd+bd
  out[b, s, f] = pred_s

Strategy: data-parallel over 8 NeuronCores (B=4096 -> 512/core); the
sequential time loop stays local per shard.  The 512-row shard is further
split into TWO independent 256-row streams whose LSTM recurrences interleave:
while stream A sits in its serial step boundary (h-mul -> recurrence matmuls
-> first gate activation), the Activation engine processes stream B's gates,
keeping the bottleneck engine (Act) near-saturated.

On-chip layout is transposed (partitions = unit index within a 128-chunk,
free dim = (unit-chunk, batch)): z per (stream, gate) is a [128, 512] PSUM
tile (= one bank; cols = 2 unit-chunks x 256 batch), activated in one wide
Act op; gate/h tiles are bf16 so DVE elementwise runs in 2x perf mode and h
feeds the next step's matmuls directly with no transposes.

Bias handling: warmup x tiles are [65, 512] with a constant ones row and the
stationary W carries a bias row, so z picks up +b inside the x-matmul and the
wide activations need no per-partition bias operand (which could not express
a bias differing between the two unit-chunk column halves).  Decode (no
x-pass) initializes each accumulation with a K=1 ones-row matmul carrying the
fused decode bias.

The autoregressive decode is algebraically fused: since
  z_{t+1} = pred_t @ W + h_t @ U + b   and   pred_t = h_t @ Wd + bd,
we precompute Ud = U + Wd@W and bdec = b + bd@W on the host, so each decode
step is a single K=256 recurrence; pred is computed off the critical path
directly in [batch, feature] layout (h as the stationary operand), so the
output path needs no transposes.
"""

import sys

for _p in ("/opt/trn_rl_repo", "/root/.axon_site/_ro/trn_rl_repo"):
    if _p not in sys.path:
        sys.path.insert(0, _p)

import numpy as np

import concourse.bacc as bacc
import concourse.mybir as mybir
import concourse.tile as tile
from concourse import bass_utils

F32 = mybir.dt.float32
BF16 = mybir.dt.bfloat16
AF = mybir.ActivationFunctionType

N_CORES = 8
F = 64          # input/output feature dim
U = 256         # lstm units
U4 = 4 * U      # gate rows
XLOOK = 3       # steps of x-staging lookahead
NXS = 6         # static x tiles
NS = 2          # batch streams per core
SB = 256        # stream batch

G_F, G_I, G_G, G_O = 0, 1, 2, 3
# column base in the 1024-wide gate row space, keras order (i, f, g, o)
GCOL = {G_I: 0, G_F: 256, G_G: 512, G_O: 768}
CONTIG = True  # emit each accumulation group's passes contiguously (debug)


def build_program(B, T, out_steps):
    """Single-core SPMD program for a batch shard of size B (=512)."""
    assert B == 512, "tile geometry is hardcoded for a 512-row shard"
    NB = B // 128

    nc = bacc.Bacc("TRN2", target_bir_lowering=False, debug=False, num_devices=1)

    xin = nc.dram_tensor("xin", [B, T, F], F32, kind="ExternalInput").ap()
    wb_d = nc.dram_tensor("wb", [F + 1, U4], BF16, kind="ExternalInput").ap()
    u2_d = nc.dram_tensor("u2", [128, 2 * U4], BF16, kind="ExternalInput").ap()
    ud2_d = nc.dram_tensor("ud2", [128, 2 * U4], BF16, kind="ExternalInput").ap()
    wdd2_d = nc.dram_tensor("wdd2", [128, 2 * F], BF16, kind="ExternalInput").ap()
    bdec_d = nc.dram_tensor("bdec", [1, U4], BF16, kind="ExternalInput").ap()
    bdrow_d = nc.dram_tensor("bdrow", [1, F], BF16, kind="ExternalInput").ap()
    ones_d = nc.dram_tensor("ones", [1, SB], BF16, kind="ExternalInput").ap()
    ident_d = nc.dram_tensor("ident", [128, 128], F32, kind="ExternalInput").ap()
    yout = nc.dram_tensor("yout", [B, out_steps, F], F32, kind="ExternalOutput").ap()

    xin_f = xin.rearrange("b t f -> b (t f)")
    yout_f = yout.rearrange("b s f -> b (s f)")

    with tile.TileContext(nc) as tc:
        import contextlib

        with contextlib.ExitStack() as ctx:
            wpool = ctx.enter_context(tc.tile_pool(name="wpool", bufs=1))
            xspool = ctx.enter_context(tc.tile_pool(name="xspool", bufs=1))
            dpool = ctx.enter_context(tc.tile_pool(name="dpool", bufs=16))
            gpool = ctx.enter_context(tc.tile_pool(name="gpool", bufs=2))
            tpool = ctx.enter_context(tc.tile_pool(name="tpool", bufs=2))
            cpool = ctx.enter_context(tc.tile_pool(name="cpool", bufs=2))
            hpool = ctx.enter_context(tc.tile_pool(name="hpool", bufs=3))
            opool = ctx.enter_context(tc.tile_pool(name="opool", bufs=4))
            zpool = ctx.enter_context(tc.tile_pool(name="zpool", bufs=6, space="PSUM"))
            aux = ctx.enter_context(tc.tile_pool(name="aux", bufs=2, space="PSUM"))

            # ---- weights / constants ----
            # (ident + wb first: step 0 needs only those; the big u2/ud2
            # transfers go after the prologue x staging, see below)
            ident = wpool.tile([128, 128], F32)
            nc.sync.dma_start(ident[:], ident_d[:])
            wb = wpool.tile([F + 1, U4], BF16)
            nc.sync.dma_start(wb[:], wb_d[:])
            ones = wpool.tile([1, SB], BF16)
            nc.sync.dma_start(ones[:], ones_d[:])
            u2 = wpool.tile([128, 2 * U4], BF16)
            ud2 = wpool.tile([128, 2 * U4], BF16)
            wdd2 = wpool.tile([128, 2 * F], BF16)
            bdec = wpool.tile([1, U4], BF16)
            bdrow = wpool.tile([1, F], BF16)

            # static x tiles: rows 0:64 = x_t^T (bf16), row 64 = ones
            # (cols st*SB:(st+1)*SB belong to stream st)
            xs = [xspool.tile([F + 1, B], BF16, name=f"xs{j}") for j in range(NXS)]
            for j in range(NXS):
                nc.gpsimd.memset(xs[j][F : F + 1, :], 1.0)

            # ---- x staging: DMA 4 batch-chunks, PE-transpose, Pool-copy ----
            def stage_x_dma(t):
                dts = []
                for bc in range(NB):
                    dt_in = dpool.tile([128, F], F32, tag="din", name=f"din{t}_{bc}")
                    nc.sync.dma_start(
                        dt_in[:],
                        xin_f[128 * bc : 128 * (bc + 1), F * t : F * (t + 1)],
                    )
                    dts.append(dt_in)
                return dts

            def stage_x_transpose(t, dts):
                xp = aux.tile([128, B], F32, tag="aux", name=f"xp{t}")
                for bc in range(NB):
                    nc.tensor.transpose(
                        xp[0:F, 128 * bc : 128 * (bc + 1)], dts[bc][:], ident[:]
                    )
                # Pool/GPSIMD cannot read PSUM on real HW; copy on DVE
                nc.vector.tensor_copy(xs[t % NXS][0:F, :], xp[0:F, :])

            # ---- per-(stream, step) PE pass emission ----
            # PSUM accumulation groups have 2KB-bank ("zero region")
            # granularity: the two column-half groups of a gate tile must run
            # SEQUENTIALLY (half 0's start..stop fully before half 1 starts).
            def emit_gate(t, st, zt, x_t, q, uw, h_prev, first):
                zq = zpool.tile([128, 2 * SB], F32, tag="z", name=f"z{t}_{st}_{q}")
                zt[q] = zq
                for hcol in (0, 1):
                    mcol = GCOL[q] + 128 * hcol
                    dst = zq[:, SB * hcol : SB * (hcol + 1)]
                    if x_t is not None:
                        nc.tensor.matmul(
                            dst, wb[:, mcol : mcol + 128],
                            x_t[:, SB * st : SB * (st + 1)],
                            start=True, stop=first,
                        )
                    else:
                        nc.tensor.matmul(
                            dst, bdec[:, mcol : mcol + 128], ones[:],
                            start=True, stop=first,
                        )
                    if not first:
                        nc.tensor.matmul(
                            dst, uw[:, mcol : mcol + 128], h_prev[:, 0:SB],
                            start=False, stop=False,
                        )
                        nc.tensor.matmul(
                            dst, uw[:, U4 + mcol : U4 + mcol + 128],
                            h_prev[:, SB : 2 * SB],
                            start=False, stop=True,
                        )

            def emit_acts(t, st, zt, c_prev, g_t):
                """Act ops f,i,g,o for one stream (tc emitted in emit_dve)."""
                nc.scalar.activation(g_t["f"][:], zt[G_F][:], AF.Sigmoid)
                nc.scalar.activation(g_t["i"][:], zt[G_I][:], AF.Sigmoid)
                nc.scalar.activation(g_t["g"][:], zt[G_G][:], AF.Tanh)
                nc.scalar.activation(g_t["o"][:], zt[G_O][:], AF.Sigmoid)

            def emit_dve(t, st, c_prev, g_t):
                """Elementwise chain + tanh(c) + h for one stream."""
                m = tpool.tile([128, 2 * SB], BF16, tag="m", name=f"m{t}_{st}")
                fc = tpool.tile([128, 2 * SB], F32, tag="fc", name=f"fc{t}_{st}")
                c_t = cpool.tile([128, 2 * SB], F32, tag="c", name=f"c{t}_{st}")
                tc_t = gpool.tile([128, 2 * SB], BF16, tag="tc", name=f"tc{t}_{st}")
                h_t = hpool.tile([128, 2 * SB], BF16, tag="h", name=f"h{t}_{st}")

                if c_prev is not None:
                    nc.vector.tensor_mul(fc[:], g_t["f"][:], c_prev[:])
                nc.vector.tensor_mul(m[:], g_t["i"][:], g_t["g"][:])
                if c_prev is not None:
                    nc.vector.tensor_add(c_t[:], fc[:], m[:])
                else:
                    nc.vector.tensor_copy(c_t[:], m[:])
                nc.scalar.activation(tc_t[:], c_t[:], AF.Tanh)
                nc.vector.tensor_mul(h_t[:], g_t["o"][:], tc_t[:])
                return h_t, c_t

            # ---- pred + output (per stream: 2 batch chunks of 128) ----
            def emit_pred_mm(s, st, h_t):
                pp = aux.tile([128, B], F32, tag="aux", name=f"pp{s}_{st}")
                for j in range(2):
                    dst = pp[:, F * j : F * (j + 1)]
                    nc.tensor.matmul(
                        dst, ones[0:1, 0:128], bdrow[:], start=True, stop=False
                    )
                    nc.tensor.matmul(
                        dst, h_t[:, 128 * j : 128 * (j + 1)], wdd2[:, 0:F],
                        start=False, stop=False,
                    )
                    nc.tensor.matmul(
                        dst, h_t[:, SB + 128 * j : SB + 128 * (j + 1)],
                        wdd2[:, F : 2 * F],
                        start=False, stop=True,
                    )
                return pp

            def emit_pred_out(s, st, pp):
                osb = opool.tile([128, 2 * F], F32, tag="ot", name=f"osb{s}_{st}")
                nc.vector.tensor_copy(osb[:], pp[:, 0 : 2 * F])
                for j in range(2):
                    bc = 2 * st + j
                    nc.sync.dma_start(
                        yout_f[128 * bc : 128 * (bc + 1), F * s : F * (s + 1)],
                        osb[:, F * j : F * (j + 1)],
                    )

            # ---- prologue: stage x for the first steps ----
            dma_q = {}
            for t in range(min(XLOOK, T)):
                dma_q[t] = stage_x_dma(t)
            # big weight transfers after the first x tiles are in flight
            nc.sync.dma_start(u2[:], u2_d[:])
            nc.sync.dma_start(ud2[:], ud2_d[:])
            nc.sync.dma_start(wdd2[:], wdd2_d[:])
            nc.sync.dma_start(bdec[:], bdec_d[:])
            nc.sync.dma_start(bdrow[:], bdrow_d[:])
            for t in range(min(XLOOK, T)):
                stage_x_transpose(t, dma_q.pop(t))
            if XLOOK < T:
                dma_q[XLOOK] = stage_x_dma(XLOOK)

            n_steps = T + (out_steps - 1)
            h_prev = [None] * NS
            c_prev = [None] * NS

            hs = {}
            for t in range(n_steps):
                warm = t < T
                x_t = xs[t % NXS] if warm else None
                uw = u2 if warm else ud2
                first = h_prev[0] is None

                # --- PE: recurrence blocks per stream ---
                zt = [dict() for _ in range(NS)]
                for st in range(NS):
                    for q in (G_F, G_I, G_G, G_O):
                        emit_gate(t, st, zt[st], x_t, q, uw, h_prev[st], first)

                # pred matmuls for the previous step's h (decode lags 1 step)
                if t >= T and (t - 1) in hs:
                    pps = [emit_pred_mm(t - T, st, hs[t - 1][st]) for st in range(NS)]

                # --- x staging for upcoming steps ---
                if t + 1 < n_steps:
                    if t + XLOOK < T and t + XLOOK in dma_q:
                        stage_x_transpose(t + XLOOK, dma_q.pop(t + XLOOK))
                    if t + XLOOK + 1 < T:
                        dma_q[t + XLOOK + 1] = stage_x_dma(t + XLOOK + 1)

                # --- Act/DVE tails, stream-interleaved ---
                g_ts = []
                for st in range(NS):
                    g_t = {
                        k: gpool.tile(
                            [128, 2 * SB], BF16, tag=f"g{k}", name=f"g{k}{t}_{st}"
                        )
                        for k in ("f", "i", "g", "o")
                    }
                    g_ts.append(g_t)
                    emit_acts(t, st, zt[st], c_prev[st], g_t)
                    h_prev[st], c_prev[st] = emit_dve(t, st, c_prev[st], g_t)

                # output DMA for lagged preds
                if t >= T and (t - 1) in hs:
                    for st in range(NS):
                        emit_pred_out(t - T, st, pps[st])
                    del hs[t - 1]
                if t >= T - 1:
                    hs[t] = list(h_prev)

            # epilogue: last pred
            for st in range(NS):
                pp = emit_pred_mm(out_steps - 1, st, hs[n_steps - 1][st])
                emit_pred_out(out_steps - 1, st, pp)

    nc.compile()
    return nc


_CACHE = {}


def _get_program(key):
    if key not in _CACHE:
        _CACHE[key] = build_program(*key)
    return _CACHE[key]


def _host_prep(W, Uk, b, Wd, bd):
    bf16 = mybir.dt.np(BF16)
    W64 = W.astype(np.float64)
    Ud = (Uk.astype(np.float64) + Wd.astype(np.float64) @ W64).astype(np.float32)
    bdec = (b.astype(np.float64) + bd.astype(np.float64) @ W64).astype(np.float32)
    wb = np.concatenate([W, b.reshape(1, -1)], axis=0)          # [65, 1024]
    u2 = np.concatenate([Uk[0:128], Uk[128:256]], axis=1)       # [128, 2048]
    ud2 = np.concatenate([Ud[0:128], Ud[128:256]], axis=1)
    wdd2 = np.concatenate([Wd[0:128], Wd[128:256]], axis=1)     # [128, 128]
    return {
        "wb": wb.astype(bf16),
        "u2": u2.astype(bf16),
        "ud2": ud2.astype(bf16),
        "wdd2": wdd2.astype(bf16),
        "bdec": bdec.reshape(1, -1).astype(bf16),
        "bdrow": bd.reshape(1, -1).astype(bf16),
        "ones": np.ones((1, SB), dtype=bf16),
        "ident": np.eye(128, dtype=np.float32),
    }


def kernel(inputs, W, U, b, Wd, bd, out_steps):
    inputs = np.asarray(inputs, dtype=np.float32)
    W = np.asarray(W, dtype=np.float32)
    U_ = np.asarray(U, dtype=np.float32)
    b_ = np.asarray(b, dtype=np.float32)
    Wd = np.asarray(Wd, dtype=np.float32)
    bd = np.asarray(bd, dtype=np.float32)
    out_steps = int(out_steps)

    B_full, T, _ = inputs.shape
    assert B_full % N_CORES == 0
    Bc = B_full // N_CORES

    nc = _get_program((Bc, T, out_steps))
    shared = _host_prep(W, U_, b_, Wd, bd)
    in_maps = [
        {"xin": np.ascontiguousarray(inputs[i * Bc : (i + 1) * Bc]), **shared}
        for i in range(N_CORES)
    ]
    res = bass_utils.run_bass_kernel_spmd(nc, in_maps, core_ids=list(range(N_CORES)))
    out = np.concatenate([res.results[i]["yout"] for i in range(N_CORES)], axis=0)
    return out


# revision 18
# speedup vs baseline: 1.3617x; 1.0593x over previous
"""LSTM warmup + autoregressive decode kernel for Trainium2 (Bass/Tile).

Reference computation (per batch row):
  h,c = 0
  for t in range(T):  h,c = LSTMstep(x_t)        # warmup over input seq
  pred0 = h @ Wd + bd
  for d in range(out_steps-1): h,c = LSTMstep(pred_d); pred_{d+1} = h@Wd+bd
  out[b, s, f] = pred_s

Strategy: data-parallel over 8 NeuronCores (B=4096 -> 512/core); the
sequential time loop stays local per shard.  The 512-row shard is further
split into TWO independent 256-row streams whose LSTM recurrences interleave:
while stream A sits in its serial step boundary (h-mul -> recurrence matmuls
-> first gate activation), the Activation engine processes stream B's gates,
keeping the bottleneck engine (Act) near-saturated.

On-chip layout is transposed (partitions = unit index within a 128-chunk,
free dim = (unit-chunk, batch)): z per (stream, gate) is a [128, 512] PSUM
tile (= one bank; cols = 2 unit-chunks x 256 batch), activated in one wide
Act op; gate/h tiles are bf16 so DVE elementwise runs in 2x perf mode and h
feeds the next step's matmuls directly with no transposes.

Bias handling: warmup x tiles are [65, 512] with a constant ones row and the
stationary W carries a bias row, so z picks up +b inside the x-matmul and the
wide activations need no per-partition bias operand (which could not express
a bias differing between the two unit-chunk column halves).  Decode (no
x-pass) initializes each accumulation with a K=1 ones-row matmul carrying the
fused decode bias.

The autoregressive decode is algebraically fused: since
  z_{t+1} = pred_t @ W + h_t @ U + b   and   pred_t = h_t @ Wd + bd,
we precompute Ud = U + Wd@W and bdec = b + bd@W on the host, so each decode
step is a single K=256 recurrence; pred is computed off the critical path
directly in [batch, feature] layout (h as the stationary operand), so the
output path needs no transposes.
"""

import sys

for _p in ("/opt/trn_rl_repo", "/root/.axon_site/_ro/trn_rl_repo"):
    if _p not in sys.path:
        sys.path.insert(0, _p)

import numpy as np

import concourse.bacc as bacc
import concourse.mybir as mybir
import concourse.tile as tile
from concourse import bass_utils

F32 = mybir.dt.float32
BF16 = mybir.dt.bfloat16
AF = mybir.ActivationFunctionType

N_CORES = 8
F = 64          # input/output feature dim
U = 256         # lstm units
U4 = 4 * U      # gate rows
XLOOK = 3       # steps of x-staging lookahead
NXS = 6         # static x tiles
NS = 2          # batch streams per core
SB = 256        # stream batch

G_F, G_I, G_G, G_O = 0, 1, 2, 3
# column base in the 1024-wide gate row space, keras order (i, f, g, o)
GCOL = {G_I: 0, G_F: 256, G_G: 512, G_O: 768}
CDT = BF16  # cell-state dtype (F32 for extra precision margin)


def build_program(B, T, out_steps):
    """Single-core SPMD program for a batch shard of size B (=512)."""
    assert B == 512, "tile geometry is hardcoded for a 512-row shard"
    NB = B // 128

    nc = bacc.Bacc("TRN2", target_bir_lowering=False, debug=False, num_devices=1)

    xin = nc.dram_tensor("xin", [B, T, F], F32, kind="ExternalInput").ap()
    wb_d = nc.dram_tensor("wb", [F + 1, U4], BF16, kind="ExternalInput").ap()
    u2_d = nc.dram_tensor("u2", [128, 2 * U4], BF16, kind="ExternalInput").ap()
    ud2_d = nc.dram_tensor("ud2", [128, 2 * U4], BF16, kind="ExternalInput").ap()
    wdd2_d = nc.dram_tensor("wdd2", [128, 2 * F], BF16, kind="ExternalInput").ap()
    bdec_d = nc.dram_tensor("bdec", [1, U4], BF16, kind="ExternalInput").ap()
    bdrow_d = nc.dram_tensor("bdrow", [1, F], BF16, kind="ExternalInput").ap()
    ones_d = nc.dram_tensor("ones", [1, SB], BF16, kind="ExternalInput").ap()
    ident_d = nc.dram_tensor("ident", [128, 128], F32, kind="ExternalInput").ap()
    yout = nc.dram_tensor("yout", [B, out_steps, F], F32, kind="ExternalOutput").ap()

    xin_f = xin.rearrange("b t f -> b (t f)")
    yout_f = yout.rearrange("b s f -> b (s f)")

    with tile.TileContext(nc) as tc:
        import contextlib

        with contextlib.ExitStack() as ctx:
            wpool = ctx.enter_context(tc.tile_pool(name="wpool", bufs=1))
            xspool = ctx.enter_context(tc.tile_pool(name="xspool", bufs=1))
            dpool = ctx.enter_context(tc.tile_pool(name="dpool", bufs=16))
            gpool = ctx.enter_context(tc.tile_pool(name="gpool", bufs=2))
            tpool = ctx.enter_context(tc.tile_pool(name="tpool", bufs=2))
            cpool = ctx.enter_context(tc.tile_pool(name="cpool", bufs=2))
            hpool = ctx.enter_context(tc.tile_pool(name="hpool", bufs=3))
            opool = ctx.enter_context(tc.tile_pool(name="opool", bufs=4))
            zpool = ctx.enter_context(tc.tile_pool(name="zpool", bufs=6, space="PSUM"))
            aux = ctx.enter_context(tc.tile_pool(name="aux", bufs=2, space="PSUM"))

            # ---- weights / constants ----
            # (ident + wb first: step 0 needs only those; the big u2/ud2
            # transfers go after the prologue x staging, see below)
            ident = wpool.tile([128, 128], F32)
            nc.sync.dma_start(ident[:], ident_d[:])
            wb = wpool.tile([F + 1, U4], BF16)
            nc.sync.dma_start(wb[:], wb_d[:])
            ones = wpool.tile([1, SB], BF16)
            nc.sync.dma_start(ones[:], ones_d[:])
            u2 = wpool.tile([128, 2 * U4], BF16)
            ud2 = wpool.tile([128, 2 * U4], BF16)
            wdd2 = wpool.tile([128, 2 * F], BF16)
            bdec = wpool.tile([1, U4], BF16)
            bdrow = wpool.tile([1, F], BF16)

            # static x tiles: rows 0:64 = x_t^T (bf16), row 64 = ones
            # (cols st*SB:(st+1)*SB belong to stream st)
            xs = [xspool.tile([F + 1, B], BF16, name=f"xs{j}") for j in range(NXS)]
            for j in range(NXS):
                nc.gpsimd.memset(xs[j][F : F + 1, :], 1.0)

            # ---- x staging: DMA 4 batch-chunks, PE-transpose, Pool-copy ----
            def stage_x_dma(t):
                dts = []
                for bc in range(NB):
                    dt_in = dpool.tile([128, F], F32, tag="din", name=f"din{t}_{bc}")
                    nc.sync.dma_start(
                        dt_in[:],
                        xin_f[128 * bc : 128 * (bc + 1), F * t : F * (t + 1)],
                    )
                    dts.append(dt_in)
                return dts

            def stage_x_transpose(t, dts):
                xp = aux.tile([128, B], F32, tag="aux", name=f"xp{t}")
                for bc in range(NB):
                    nc.tensor.transpose(
                        xp[0:F, 128 * bc : 128 * (bc + 1)], dts[bc][:], ident[:]
                    )
                # Pool/GPSIMD cannot read PSUM on real HW; copy on DVE
                nc.vector.tensor_copy(xs[t % NXS][0:F, :], xp[0:F, :])

            # ---- per-(stream, step) PE pass emission ----
            # PSUM accumulation groups have 2KB-bank ("zero region")
            # granularity: the two column-half groups of a gate tile must run
            # SEQUENTIALLY (half 0's start..stop fully before half 1 starts).
            def emit_gate(t, st, zt, x_t, q, uw, h_prev, first):
                zq = zpool.tile([128, 2 * SB], F32, tag="z", name=f"z{t}_{st}_{q}")
                zt[q] = zq
                for hcol in (0, 1):
                    mcol = GCOL[q] + 128 * hcol
                    dst = zq[:, SB * hcol : SB * (hcol + 1)]
                    if x_t is not None:
                        nc.tensor.matmul(
                            dst, wb[:, mcol : mcol + 128],
                            x_t[:, SB * st : SB * (st + 1)],
                            start=True, stop=first,
                        )
                    else:
                        nc.tensor.matmul(
                            dst, bdec[:, mcol : mcol + 128], ones[:],
                            start=True, stop=first,
                        )
                    if not first:
                        nc.tensor.matmul(
                            dst, uw[:, mcol : mcol + 128], h_prev[:, 0:SB],
                            start=False, stop=False,
                        )
                        nc.tensor.matmul(
                            dst, uw[:, U4 + mcol : U4 + mcol + 128],
                            h_prev[:, SB : 2 * SB],
                            start=False, stop=True,
                        )

            def emit_acts(t, st, zt, c_prev, g_t):
                """Act ops f,i,g,o for one stream (tc emitted in emit_dve)."""
                nc.scalar.activation(g_t["f"][:], zt[G_F][:], AF.Sigmoid)
                nc.scalar.activation(g_t["i"][:], zt[G_I][:], AF.Sigmoid)
                nc.scalar.activation(g_t["g"][:], zt[G_G][:], AF.Tanh)
                nc.scalar.activation(g_t["o"][:], zt[G_O][:], AF.Sigmoid)

            def emit_dve(t, st, c_prev, g_t):
                """Elementwise chain + tanh(c) + h for one stream."""
                m = tpool.tile([128, 2 * SB], BF16, tag="m", name=f"m{t}_{st}")
                fc = tpool.tile([128, 2 * SB], CDT, tag="fc", name=f"fc{t}_{st}")
                c_t = cpool.tile([128, 2 * SB], CDT, tag="c", name=f"c{t}_{st}")
                tc_t = gpool.tile([128, 2 * SB], BF16, tag="tc", name=f"tc{t}_{st}")
                h_t = hpool.tile([128, 2 * SB], BF16, tag="h", name=f"h{t}_{st}")

                if c_prev is not None:
                    nc.vector.tensor_mul(fc[:], g_t["f"][:], c_prev[:])
                nc.vector.tensor_mul(m[:], g_t["i"][:], g_t["g"][:])
                if c_prev is not None:
                    nc.vector.tensor_add(c_t[:], fc[:], m[:])
                else:
                    nc.vector.tensor_copy(c_t[:], m[:])
                nc.scalar.activation(tc_t[:], c_t[:], AF.Tanh)
                nc.vector.tensor_mul(h_t[:], g_t["o"][:], tc_t[:])
                return h_t, c_t

            # ---- pred + output (per stream: 2 batch chunks of 128) ----
            def emit_pred_mm(s, st, h_t):
                pp = aux.tile([128, B], F32, tag="aux", name=f"pp{s}_{st}")
                for j in range(2):
                    dst = pp[:, F * j : F * (j + 1)]
                    nc.tensor.matmul(
                        dst, ones[0:1, 0:128], bdrow[:], start=True, stop=False
                    )
                    nc.tensor.matmul(
                        dst, h_t[:, 128 * j : 128 * (j + 1)], wdd2[:, 0:F],
                        start=False, stop=False,
                    )
                    nc.tensor.matmul(
                        dst, h_t[:, SB + 128 * j : SB + 128 * (j + 1)],
                        wdd2[:, F : 2 * F],
                        start=False, stop=True,
                    )
                return pp

            def emit_pred_out(s, st, pp):
                osb = opool.tile([128, 2 * F], F32, tag="ot", name=f"osb{s}_{st}")
                nc.vector.tensor_copy(osb[:], pp[:, 0 : 2 * F])
                for j in range(2):
                    bc = 2 * st + j
                    nc.sync.dma_start(
                        yout_f[128 * bc : 128 * (bc + 1), F * s : F * (s + 1)],
                        osb[:, F * j : F * (j + 1)],
                    )

            # ---- prologue: stage x for the first steps ----
            dma_q = {}
            for t in range(min(XLOOK, T)):
                dma_q[t] = stage_x_dma(t)
            # big weight transfers after the first x tiles are in flight
            nc.sync.dma_start(u2[:], u2_d[:])
            nc.sync.dma_start(ud2[:], ud2_d[:])
            nc.sync.dma_start(wdd2[:], wdd2_d[:])
            nc.sync.dma_start(bdec[:], bdec_d[:])
            nc.sync.dma_start(bdrow[:], bdrow_d[:])
            for t in range(min(XLOOK, T)):
                stage_x_transpose(t, dma_q.pop(t))
            if XLOOK < T:
                dma_q[XLOOK] = stage_x_dma(XLOOK)

            n_steps = T + (out_steps - 1)
            h_prev = [None] * NS
            c_prev = [None] * NS

            hs = {}
            for t in range(n_steps):
                warm = t < T
                x_t = xs[t % NXS] if warm else None
                uw = u2 if warm else ud2
                first = h_prev[0] is None

                # --- PE: recurrence blocks per stream ---
                zt = [dict() for _ in range(NS)]
                for st in range(NS):
                    for q in (G_F, G_I, G_G, G_O):
                        emit_gate(t, st, zt[st], x_t, q, uw, h_prev[st], first)

                # pred matmuls for the previous step's h (decode lags 1 step)
                if t >= T and (t - 1) in hs:
                    pps = [emit_pred_mm(t - T, st, hs[t - 1][st]) for st in range(NS)]

                # --- x staging for upcoming steps ---
                if t + 1 < n_steps:
                    if t + XLOOK < T and t + XLOOK in dma_q:
                        stage_x_transpose(t + XLOOK, dma_q.pop(t + XLOOK))
                    if t + XLOOK + 1 < T:
                        dma_q[t + XLOOK + 1] = stage_x_dma(t + XLOOK + 1)

                # --- Act/DVE tails, stream-interleaved ---
                g_ts = []
                for st in range(NS):
                    g_t = {
                        k: gpool.tile(
                            [128, 2 * SB], BF16, tag=f"g{k}", name=f"g{k}{t}_{st}"
                        )
                        for k in ("f", "i", "g", "o")
                    }
                    g_ts.append(g_t)
                    emit_acts(t, st, zt[st], c_prev[st], g_t)
                    h_prev[st], c_prev[st] = emit_dve(t, st, c_prev[st], g_t)

                # output DMA for lagged preds
                if t >= T and (t - 1) in hs:
                    for st in range(NS):
                        emit_pred_out(t - T, st, pps[st])
                    del hs[t - 1]
                if t >= T - 1:
                    hs[t] = list(h_prev)

            # epilogue: last pred
            for st in range(NS):
                pp = emit_pred_mm(out_steps - 1, st, hs[n_steps - 1][st])
                emit_pred_out(out_steps - 1, st, pp)

    nc.compile()
    return nc


_CACHE = {}


def _get_program(key):
    if key not in _CACHE:
        _CACHE[key] = build_program(*key)
    return _CACHE[key]


def _host_prep(W, Uk, b, Wd, bd):
    bf16 = mybir.dt.np(BF16)
    W64 = W.astype(np.float64)
    Ud = (Uk.astype(np.float64) + Wd.astype(np.float64) @ W64).astype(np.float32)
    bdec = (b.astype(np.float64) + bd.astype(np.float64) @ W64).astype(np.float32)
    wb = np.concatenate([W, b.reshape(1, -1)], axis=0)          # [65, 1024]
    u2 = np.concatenate([Uk[0:128], Uk[128:256]], axis=1)       # [128, 2048]
    ud2 = np.concatenate([Ud[0:128], Ud[128:256]], axis=1)
    wdd2 = np.concatenate([Wd[0:128], Wd[128:256]], axis=1)     # [128, 128]
    return {
        "wb": wb.astype(bf16),
        "u2": u2.astype(bf16),
        "ud2": ud2.astype(bf16),
        "wdd2": wdd2.astype(bf16),
        "bdec": bdec.reshape(1, -1).astype(bf16),
        "bdrow": bd.reshape(1, -1).astype(bf16),
        "ones": np.ones((1, SB), dtype=bf16),
        "ident": np.eye(128, dtype=np.float32),
    }


def kernel(inputs, W, U, b, Wd, bd, out_steps):
    inputs = np.asarray(inputs, dtype=np.float32)
    W = np.asarray(W, dtype=np.float32)
    U_ = np.asarray(U, dtype=np.float32)
    b_ = np.asarray(b, dtype=np.float32)
    Wd = np.asarray(Wd, dtype=np.float32)
    bd = np.asarray(bd, dtype=np.float32)
    out_steps = int(out_steps)

    B_full, T, _ = inputs.shape
    assert B_full % N_CORES == 0
    Bc = B_full // N_CORES

    nc = _get_program((Bc, T, out_steps))
    shared = _host_prep(W, U_, b_, Wd, bd)
    in_maps = [
        {"xin": np.ascontiguousarray(inputs[i * Bc : (i + 1) * Bc]), **shared}
        for i in range(N_CORES)
    ]
    res = bass_utils.run_bass_kernel_spmd(nc, in_maps, core_ids=list(range(N_CORES)))
    out = np.concatenate([res.results[i]["yout"] for i in range(N_CORES)], axis=0)
    return out


# revision 23
# speedup vs baseline: 1.3677x; 1.0044x over previous
"""LSTM warmup + autoregressive decode kernel for Trainium2 (Bass/Tile).

Reference computation (per batch row):
  h,c = 0
  for t in range(T):  h,c = LSTMstep(x_t)        # warmup over input seq
  pred0 = h @ Wd + bd
  for d in range(out_steps-1): h,c = LSTMstep(pred_d); pred_{d+1} = h@Wd+bd
  out[b, s, f] = pred_s

Strategy: data-parallel over 8 NeuronCores (B=4096 -> 512/core); the
sequential time loop stays local per shard.  The 512-row shard is further
split into TWO independent 256-row streams whose LSTM recurrences interleave:
while stream A sits in its serial step boundary (h-mul -> recurrence matmuls
-> first gate activation), the Activation engine processes stream B's gates,
keeping the bottleneck engine (Act) near-saturated.

On-chip layout is transposed (partitions = unit index within a 128-chunk,
free dim = (unit-chunk, batch)): z per (stream, gate) is a [128, 512] PSUM
tile (= one bank; cols = 2 unit-chunks x 256 batch), activated in one wide
Act op; gate/h tiles are bf16 so DVE elementwise runs in 2x perf mode and h
feeds the next step's matmuls directly with no transposes.

Bias handling: warmup x tiles are [65, 512] with a constant ones row and the
stationary W carries a bias row, so z picks up +b inside the x-matmul and the
wide activations need no per-partition bias operand (which could not express
a bias differing between the two unit-chunk column halves).  Decode (no
x-pass) initializes each accumulation with a K=1 ones-row matmul carrying the
fused decode bias.

The autoregressive decode is algebraically fused: since
  z_{t+1} = pred_t @ W + h_t @ U + b   and   pred_t = h_t @ Wd + bd,
we precompute Ud = U + Wd@W and bdec = b + bd@W on the host, so each decode
step is a single K=256 recurrence; pred is computed off the critical path
directly in [batch, feature] layout (h as the stationary operand), so the
output path needs no transposes.
"""

import sys

for _p in ("/opt/trn_rl_repo", "/root/.axon_site/_ro/trn_rl_repo"):
    if _p not in sys.path:
        sys.path.insert(0, _p)

import numpy as np

import concourse.bacc as bacc
import concourse.mybir as mybir
import concourse.tile as tile
from concourse import bass_utils

F32 = mybir.dt.float32
BF16 = mybir.dt.bfloat16
AF = mybir.ActivationFunctionType

N_CORES = 8
F = 64          # input/output feature dim
U = 256         # lstm units
U4 = 4 * U      # gate rows
XLOOK = 3       # steps of x-staging lookahead
NXS = 6         # static x tiles
NS = 2          # batch streams per core
SB = 256        # stream batch

G_F, G_I, G_G, G_O = 0, 1, 2, 3
# column base in the 1024-wide gate row space, keras order (i, f, g, o)
GCOL = {G_I: 0, G_F: 256, G_G: 512, G_O: 768}
CDT = BF16  # cell-state dtype (F32 for extra precision margin)


def build_program(B, T, out_steps):
    """Single-core SPMD program for a batch shard of size B (=512)."""
    assert B == 512, "tile geometry is hardcoded for a 512-row shard"
    NB = B // 128

    nc = bacc.Bacc("TRN2", target_bir_lowering=False, debug=False, num_devices=1)

    xin = nc.dram_tensor("xin", [B, T, F], F32, kind="ExternalInput").ap()
    wb_d = nc.dram_tensor("wb", [F + 1, U4], BF16, kind="ExternalInput").ap()
    u2_d = nc.dram_tensor("u2", [128, 2 * U4], BF16, kind="ExternalInput").ap()
    ud2_d = nc.dram_tensor("ud2", [128, 2 * U4], BF16, kind="ExternalInput").ap()
    wdd2_d = nc.dram_tensor("wdd2", [128, 2 * F], BF16, kind="ExternalInput").ap()
    bdec_d = nc.dram_tensor("bdec", [1, U4], BF16, kind="ExternalInput").ap()
    bdrow_d = nc.dram_tensor("bdrow", [1, F], BF16, kind="ExternalInput").ap()
    ones_d = nc.dram_tensor("ones", [1, SB], BF16, kind="ExternalInput").ap()
    ident_d = nc.dram_tensor("ident", [128, 128], F32, kind="ExternalInput").ap()
    yout = nc.dram_tensor("yout", [B, out_steps, F], F32, kind="ExternalOutput").ap()

    xin_f = xin.rearrange("b t f -> b (t f)")
    xin_c = xin.rearrange("(c p) t f -> p c (t f)", c=4)   # [128, 4, T*F]
    yout_f = yout.rearrange("b s f -> b (s f)")
    yout_c = yout.rearrange("(c p) s f -> p c (s f)", c=4)  # [128, 4, S*F]

    with tile.TileContext(nc) as tc:
        import contextlib

        with contextlib.ExitStack() as ctx:
            wpool = ctx.enter_context(tc.tile_pool(name="wpool", bufs=1))
            xspool = ctx.enter_context(tc.tile_pool(name="xspool", bufs=1))
            dpool = ctx.enter_context(tc.tile_pool(name="dpool", bufs=16))
            gpool = ctx.enter_context(tc.tile_pool(name="gpool", bufs=2))
            tpool = ctx.enter_context(tc.tile_pool(name="tpool", bufs=2))
            cpool = ctx.enter_context(tc.tile_pool(name="cpool", bufs=2))
            hpool = ctx.enter_context(tc.tile_pool(name="hpool", bufs=3))
            opool = ctx.enter_context(tc.tile_pool(name="opool", bufs=4))
            zpool = ctx.enter_context(tc.tile_pool(name="zpool", bufs=6, space="PSUM"))
            aux = ctx.enter_context(tc.tile_pool(name="aux", bufs=2, space="PSUM"))

            # ---- weights / constants ----
            # (ident + wb first: step 0 needs only those; the big u2/ud2
            # transfers go after the prologue x staging, see below)
            ident = wpool.tile([128, 128], F32)
            nc.sync.dma_start(ident[:], ident_d[:])
            wb = wpool.tile([F + 1, U4], BF16)
            ones = wpool.tile([1, SB], BF16)
            u2 = wpool.tile([128, 2 * U4], BF16)
            ud2 = wpool.tile([128, 2 * U4], BF16)
            wdd2 = wpool.tile([128, 2 * F], BF16)
            bdec = wpool.tile([1, U4], BF16)
            bdrow = wpool.tile([1, F], BF16)

            # static x tiles: rows 0:64 = x_t^T (bf16), row 64 = ones
            # (cols st*SB:(st+1)*SB belong to stream st)
            xs = [xspool.tile([F + 1, B], BF16, name=f"xs{j}") for j in range(NXS)]
            for j in range(NXS):
                nc.gpsimd.memset(xs[j][F : F + 1, :], 1.0)

            # ---- x staging: DMA 4 batch-chunks, PE-transpose, Pool-copy ----
            def stage_x_dma(t):
                # all 4 batch-chunks of step t in one DMA: dt[p, bc*F+f]
                dt_in = dpool.tile([128, NB * F], F32, tag="din", name=f"din{t}")
                nc.sync.dma_start(
                    dt_in[:].rearrange("p (c f) -> p c f", c=NB),
                    xin_c[:, :, F * t : F * (t + 1)],
                )
                return dt_in

            def stage_x_transpose(t, dt_in):
                xp = aux.tile([128, B], F32, tag="aux", name=f"xp{t}")
                for bc in range(NB):
                    nc.tensor.transpose(
                        xp[0:F, 128 * bc : 128 * (bc + 1)],
                        dt_in[:, F * bc : F * (bc + 1)], ident[:],
                    )
                # Pool/GPSIMD cannot read PSUM on real HW; copy on DVE
                nc.vector.tensor_copy(xs[t % NXS][0:F, :], xp[0:F, :])

            # ---- per-(stream, step) PE pass emission ----
            # PSUM accumulation groups have 2KB-bank ("zero region")
            # granularity: the two column-half groups of a gate tile must run
            # SEQUENTIALLY (half 0's start..stop fully before half 1 starts).
            def emit_gate(t, st, zt, x_t, q, uw, h_prev, first):
                zq = zpool.tile([128, 2 * SB], F32, tag="z", name=f"z{t}_{st}_{q}")
                zt[q] = zq
                for hcol in (0, 1):
                    mcol = GCOL[q] + 128 * hcol
                    dst = zq[:, SB * hcol : SB * (hcol + 1)]
                    if x_t is not None:
                        nc.tensor.matmul(
                            dst, wb[:, mcol : mcol + 128],
                            x_t[:, SB * st : SB * (st + 1)],
                            start=True, stop=first,
                        )
                    else:
                        nc.tensor.matmul(
                            dst, bdec[:, mcol : mcol + 128], ones[:],
                            start=True, stop=first,
                        )
                    if not first:
                        nc.tensor.matmul(
                            dst, uw[:, mcol : mcol + 128], h_prev[:, 0:SB],
                            start=False, stop=False,
                        )
                        nc.tensor.matmul(
                            dst, uw[:, U4 + mcol : U4 + mcol + 128],
                            h_prev[:, SB : 2 * SB],
                            start=False, stop=True,
                        )

            def emit_acts(t, st, zt, c_prev, g_t):
                """Act ops f,i,g,o for one stream (tc emitted in emit_dve)."""
                nc.scalar.activation(g_t["f"][:], zt[G_F][:], AF.Sigmoid)
                nc.scalar.activation(g_t["i"][:], zt[G_I][:], AF.Sigmoid)
                nc.scalar.activation(g_t["g"][:], zt[G_G][:], AF.Tanh)
                nc.scalar.activation(g_t["o"][:], zt[G_O][:], AF.Sigmoid)

            def emit_dve(t, st, c_prev, g_t):
                """Elementwise chain + tanh(c) + h for one stream."""
                m = tpool.tile([128, 2 * SB], BF16, tag="m", name=f"m{t}_{st}")
                fc = tpool.tile([128, 2 * SB], CDT, tag="fc", name=f"fc{t}_{st}")
                c_t = cpool.tile([128, 2 * SB], CDT, tag="c", name=f"c{t}_{st}")
                tc_t = gpool.tile([128, 2 * SB], BF16, tag="tc", name=f"tc{t}_{st}")
                h_t = hpool.tile([128, 2 * SB], BF16, tag="h", name=f"h{t}_{st}")

                if c_prev is not None:
                    nc.vector.tensor_mul(fc[:], g_t["f"][:], c_prev[:])
                nc.vector.tensor_mul(m[:], g_t["i"][:], g_t["g"][:])
                if c_prev is not None:
                    nc.vector.tensor_add(c_t[:], fc[:], m[:])
                else:
                    nc.vector.tensor_copy(c_t[:], m[:])
                nc.scalar.activation(tc_t[:], c_t[:], AF.Tanh)
                nc.vector.tensor_mul(h_t[:], g_t["o"][:], tc_t[:])
                return h_t, c_t

            # ---- pred + output (per stream: 2 batch chunks of 128) ----
            def emit_pred_mm(s, st, h_t):
                pp = aux.tile([128, B], F32, tag="aux", name=f"pp{s}_{st}")
                for j in range(2):
                    dst = pp[:, F * j : F * (j + 1)]
                    nc.tensor.matmul(
                        dst, ones[0:1, 0:128], bdrow[:], start=True, stop=False
                    )
                    nc.tensor.matmul(
                        dst, h_t[:, 128 * j : 128 * (j + 1)], wdd2[:, 0:F],
                        start=False, stop=False,
                    )
                    nc.tensor.matmul(
                        dst, h_t[:, SB + 128 * j : SB + 128 * (j + 1)],
                        wdd2[:, F : 2 * F],
                        start=False, stop=True,
                    )
                return pp

            def emit_pred_out(s, st, pp):
                osb = opool.tile([128, 2 * F], F32, tag="ot", name=f"osb{s}_{st}")
                nc.vector.tensor_copy(osb[:], pp[:, 0 : 2 * F])
                nc.sync.dma_start(
                    yout_c[:, 2 * st : 2 * st + 2, F * s : F * (s + 1)],
                    osb[:].rearrange("p (c f) -> p c f", c=2),
                )

            # ---- prologue: stage x for the first steps ----
            dma_q = {}
            dma_q[0] = stage_x_dma(0)
            nc.sync.dma_start(wb[:], wb_d[:])
            nc.sync.dma_start(ones[:], ones_d[:])
            for t in range(1, min(XLOOK, T)):
                dma_q[t] = stage_x_dma(t)
            # big weight transfers after the first x tiles are in flight
            nc.sync.dma_start(u2[:], u2_d[:])
            nc.sync.dma_start(ud2[:], ud2_d[:])
            nc.sync.dma_start(wdd2[:], wdd2_d[:])
            nc.sync.dma_start(bdec[:], bdec_d[:])
            nc.sync.dma_start(bdrow[:], bdrow_d[:])
            for t in range(min(XLOOK, T)):
                stage_x_transpose(t, dma_q.pop(t))
            if XLOOK < T:
                dma_q[XLOOK] = stage_x_dma(XLOOK)

            n_steps = T + (out_steps - 1)
            h_prev = [None] * NS
            c_prev = [None] * NS

            hs = {}
            for t in range(n_steps):
                warm = t < T
                x_t = xs[t % NXS] if warm else None
                uw = u2 if warm else ud2
                first = h_prev[0] is None

                # --- PE: recurrence blocks per stream ---
                zt = [dict() for _ in range(NS)]
                for st in range(NS):
                    for q in (G_F, G_I, G_G, G_O):
                        emit_gate(t, st, zt[st], x_t, q, uw, h_prev[st], first)

                # pred matmuls for the previous step's h (decode lags 1 step)
                if t >= T and (t - 1) in hs:
                    pps = [emit_pred_mm(t - T, st, hs[t - 1][st]) for st in range(NS)]

                # --- x staging for upcoming steps ---
                if t + 1 < n_steps:
                    if t + XLOOK < T and t + XLOOK in dma_q:
                        stage_x_transpose(t + XLOOK, dma_q.pop(t + XLOOK))
                    if t + XLOOK + 1 < T:
                        dma_q[t + XLOOK + 1] = stage_x_dma(t + XLOOK + 1)

                # --- Act/DVE tails, stream-interleaved ---
                g_ts = []
                for st in range(NS):
                    g_t = {
                        k: gpool.tile(
                            [128, 2 * SB], BF16, tag=f"g{k}", name=f"g{k}{t}_{st}"
                        )
                        for k in ("f", "i", "g", "o")
                    }
                    g_ts.append(g_t)
                    emit_acts(t, st, zt[st], c_prev[st], g_t)
                    h_prev[st], c_prev[st] = emit_dve(t, st, c_prev[st], g_t)

                # output DMA for lagged preds
                if t >= T and (t - 1) in hs:
                    for st in range(NS):
                        emit_pred_out(t - T, st, pps[st])
                    del hs[t - 1]
                if t >= T - 1:
                    hs[t] = list(h_prev)

            # epilogue: last pred
            for st in range(NS):
                pp = emit_pred_mm(out_steps - 1, st, hs[n_steps - 1][st])
                emit_pred_out(out_steps - 1, st, pp)

    nc.compile()
    return nc


_CACHE = {}


def _get_program(key):
    if key not in _CACHE:
        _CACHE[key] = build_program(*key)
    return _CACHE[key]


def _host_prep(W, Uk, b, Wd, bd):
    bf16 = mybir.dt.np(BF16)
    W64 = W.astype(np.float64)
    Ud = (Uk.astype(np.float64) + Wd.astype(np.float64) @ W64).astype(np.float32)
    bdec = (b.astype(np.float64) + bd.astype(np.float64) @ W64).astype(np.float32)
    wb = np.concatenate([W, b.reshape(1, -1)], axis=0)          # [65, 1024]
    u2 = np.concatenate([Uk[0:128], Uk[128:256]], axis=1)       # [128, 2048]
    ud2 = np.concatenate([Ud[0:128], Ud[128:256]], axis=1)
    wdd2 = np.concatenate([Wd[0:128], Wd[128:256]], axis=1)     # [128, 128]
    return {
        "wb": wb.astype(bf16),
        "u2": u2.astype(bf16),
        "ud2": ud2.astype(bf16),
        "wdd2": wdd2.astype(bf16),
        "bdec": bdec.reshape(1, -1).astype(bf16),
        "bdrow": bd.reshape(1, -1).astype(bf16),
        "ones": np.ones((1, SB), dtype=bf16),
        "ident": np.eye(128, dtype=np.float32),
    }


def kernel(inputs, W, U, b, Wd, bd, out_steps):
    inputs = np.asarray(inputs, dtype=np.float32)
    W = np.asarray(W, dtype=np.float32)
    U_ = np.asarray(U, dtype=np.float32)
    b_ = np.asarray(b, dtype=np.float32)
    Wd = np.asarray(Wd, dtype=np.float32)
    bd = np.asarray(bd, dtype=np.float32)
    out_steps = int(out_steps)

    B_full, T, _ = inputs.shape
    assert B_full % N_CORES == 0
    Bc = B_full // N_CORES

    nc = _get_program((Bc, T, out_steps))
    shared = _host_prep(W, U_, b_, Wd, bd)
    in_maps = [
        {"xin": np.ascontiguousarray(inputs[i * Bc : (i + 1) * Bc]), **shared}
        for i in range(N_CORES)
    ]
    res = bass_utils.run_bass_kernel_spmd(nc, in_maps, core_ids=list(range(N_CORES)))
    out = np.concatenate([res.results[i]["yout"] for i in range(N_CORES)], axis=0)
    return out


# revision 36
# speedup vs baseline: 1.3819x; 1.0104x over previous
"""LSTM warmup + autoregressive decode kernel for Trainium2 (Bass/Tile).

Reference computation (per batch row):
  h,c = 0
  for t in range(T):  h,c = LSTMstep(x_t)        # warmup over input seq
  pred0 = h @ Wd + bd
  for d in range(out_steps-1): h,c = LSTMstep(pred_d); pred_{d+1} = h@Wd+bd
  out[b, s, f] = pred_s

Strategy: data-parallel over 8 NeuronCores (B=4096 -> 512/core); the
sequential time loop stays local per shard.  The 512-row shard is further
split into TWO independent 256-row streams whose LSTM recurrences interleave:
while stream A sits in its serial step boundary (h-mul -> recurrence matmuls
-> first gate activation), the Activation engine processes stream B's gates,
keeping the bottleneck engine (Act) near-saturated.

On-chip layout is transposed (partitions = unit index within a 128-chunk,
free dim = (unit-chunk, batch)): z per (stream, gate) is a [128, 512] PSUM
tile (= one bank; cols = 2 unit-chunks x 256 batch), activated in one wide
Act op; gate/h tiles are bf16 so DVE elementwise runs in 2x perf mode and h
feeds the next step's matmuls directly with no transposes.

Bias handling: warmup x tiles are [65, 512] with a constant ones row and the
stationary W carries a bias row, so z picks up +b inside the x-matmul and the
wide activations need no per-partition bias operand (which could not express
a bias differing between the two unit-chunk column halves).  Decode (no
x-pass) initializes each accumulation with a K=1 ones-row matmul carrying the
fused decode bias.

The autoregressive decode is algebraically fused: since
  z_{t+1} = pred_t @ W + h_t @ U + b   and   pred_t = h_t @ Wd + bd,
we precompute Ud = U + Wd@W and bdec = b + bd@W on the host, so each decode
step is a single K=256 recurrence; pred is computed off the critical path
directly in [batch, feature] layout (h as the stationary operand), so the
output path needs no transposes.
"""

import sys

for _p in ("/opt/trn_rl_repo", "/root/.axon_site/_ro/trn_rl_repo"):
    if _p not in sys.path:
        sys.path.insert(0, _p)

import numpy as np

import concourse.bacc as bacc
import concourse.mybir as mybir
import concourse.tile as tile
from concourse import bass_utils

F32 = mybir.dt.float32
BF16 = mybir.dt.bfloat16
AF = mybir.ActivationFunctionType

N_CORES = 8
F = 64          # input/output feature dim
U = 256         # lstm units
U4 = 4 * U      # gate rows
XLOOK = 3       # steps of x-staging lookahead
NXS = 6         # static x tiles
NS = 2          # batch streams per core
SB = 256        # stream batch

G_F, G_I, G_G, G_O = 0, 1, 2, 3
# column base in the 1024-wide gate row space, keras order (i, f, g, o)
GCOL = {G_I: 0, G_F: 256, G_G: 512, G_O: 768}
CDT = BF16  # cell-state dtype (F32 for extra precision margin)


def build_program(B, T, out_steps):
    """Single-core SPMD program for a batch shard of size B (=512)."""
    assert B == 512, "tile geometry is hardcoded for a 512-row shard"
    NB = B // 128

    nc = bacc.Bacc("TRN2", target_bir_lowering=False, debug=False, num_devices=1)

    xin = nc.dram_tensor("xin", [B, T, F], F32, kind="ExternalInput").ap()
    wb_d = nc.dram_tensor("wb", [F + 1, U4], BF16, kind="ExternalInput").ap()
    u2_d = nc.dram_tensor("u2", [128, 2 * U4], BF16, kind="ExternalInput").ap()
    ud2_d = nc.dram_tensor("ud2", [128, 2 * U4], BF16, kind="ExternalInput").ap()
    wdd2_d = nc.dram_tensor("wdd2", [128, 2 * F], BF16, kind="ExternalInput").ap()
    bdec_d = nc.dram_tensor("bdec", [1, U4], BF16, kind="ExternalInput").ap()
    bdrow_d = nc.dram_tensor("bdrow", [1, F], BF16, kind="ExternalInput").ap()
    ones_d = nc.dram_tensor("ones", [1, SB], BF16, kind="ExternalInput").ap()
    ident_d = nc.dram_tensor("ident", [128, 128], F32, kind="ExternalInput").ap()
    yout = nc.dram_tensor("yout", [B, out_steps, F], F32, kind="ExternalOutput").ap()

    xin_f = xin.rearrange("b t f -> b (t f)")
    xin_c = xin.rearrange("(c p) t f -> p c (t f)", c=4)   # [128, 4, T*F]
    yout_f = yout.rearrange("b s f -> b (s f)")
    yout_c = yout.rearrange("(c p) s f -> p c (s f)", c=4)  # [128, 4, S*F]

    with tile.TileContext(nc) as tc:
        import contextlib

        with contextlib.ExitStack() as ctx:
            wpool = ctx.enter_context(tc.tile_pool(name="wpool", bufs=1))
            xspool = ctx.enter_context(tc.tile_pool(name="xspool", bufs=1))
            dpool = ctx.enter_context(tc.tile_pool(name="dpool", bufs=16))
            gpool = ctx.enter_context(tc.tile_pool(name="gpool", bufs=2))
            tpool = ctx.enter_context(tc.tile_pool(name="tpool", bufs=2))
            cpool = ctx.enter_context(tc.tile_pool(name="cpool", bufs=2))
            hpool = ctx.enter_context(tc.tile_pool(name="hpool", bufs=3))
            opool = ctx.enter_context(tc.tile_pool(name="opool", bufs=4))
            zpool = ctx.enter_context(tc.tile_pool(name="zpool", bufs=6, space="PSUM"))
            aux = ctx.enter_context(tc.tile_pool(name="aux", bufs=2, space="PSUM"))

            # ---- weights / constants ----
            # (ident + wb first: step 0 needs only those; the big u2/ud2
            # transfers go after the prologue x staging, see below)
            ident = wpool.tile([128, 128], F32)
            wb = wpool.tile([F + 1, U4], BF16)
            ones = wpool.tile([1, SB], BF16)
            u2 = wpool.tile([128, 2 * U4], BF16)
            ud2 = wpool.tile([128, 2 * U4], BF16)
            wdd2 = wpool.tile([128, 2 * F], BF16)
            bdec = wpool.tile([1, U4], BF16)
            bdrow = wpool.tile([1, F], BF16)

            # static x tiles: rows 0:64 = x_t^T (bf16), row 64 = ones
            # (cols st*SB:(st+1)*SB belong to stream st)
            xs = [xspool.tile([F + 1, B], BF16, name=f"xs{j}") for j in range(NXS)]
            for j in range(NXS):
                nc.gpsimd.memset(xs[j][F : F + 1, :], 1.0)

            # ---- x staging: DMA 4 batch-chunks, PE-transpose, Pool-copy ----
            def stage_x_dma(t):
                # all 4 batch-chunks of step t in one DMA: dt[p, bc*F+f]
                dt_in = dpool.tile([128, NB * F], F32, tag="din", name=f"din{t}")
                nc.sync.dma_start(
                    dt_in[:].rearrange("p (c f) -> p c f", c=NB),
                    xin_c[:, :, F * t : F * (t + 1)],
                )
                return dt_in

            def stage_x_transpose(t, dt_in):
                xp = aux.tile([128, B], F32, tag="aux", name=f"xp{t}")
                for bc in range(NB):
                    nc.tensor.transpose(
                        xp[0:F, 128 * bc : 128 * (bc + 1)],
                        dt_in[:, F * bc : F * (bc + 1)], ident[:],
                    )
                # Pool/GPSIMD cannot read PSUM on real HW; copy on DVE
                nc.vector.tensor_copy(xs[t % NXS][0:F, :], xp[0:F, :])

            # ---- per-(stream, step) PE pass emission ----
            # PSUM accumulation groups have 2KB-bank ("zero region")
            # granularity: the two column-half groups of a gate tile must run
            # SEQUENTIALLY (half 0's start..stop fully before half 1 starts).
            def emit_gate(t, st, zt, x_t, q, uw, h_prev, first):
                zq = zpool.tile([128, 2 * SB], F32, tag="z", name=f"z{t}_{st}_{q}")
                zt[q] = zq
                for hcol in (0, 1):
                    mcol = GCOL[q] + 128 * hcol
                    dst = zq[:, SB * hcol : SB * (hcol + 1)]
                    if x_t is not None:
                        nc.tensor.matmul(
                            dst, wb[:, mcol : mcol + 128],
                            x_t[:, SB * st : SB * (st + 1)],
                            start=True, stop=first,
                        )
                    else:
                        nc.tensor.matmul(
                            dst, bdec[:, mcol : mcol + 128], ones[:],
                            start=True, stop=first,
                        )
                    if not first:
                        nc.tensor.matmul(
                            dst, uw[:, mcol : mcol + 128], h_prev[:, 0:SB],
                            start=False, stop=False,
                        )
                        nc.tensor.matmul(
                            dst, uw[:, U4 + mcol : U4 + mcol + 128],
                            h_prev[:, SB : 2 * SB],
                            start=False, stop=True,
                        )

            def emit_acts(t, st, zt, c_prev, g_t):
                """Act ops f,i,g,o for one stream (tc emitted in emit_dve)."""
                nc.scalar.activation(g_t["f"][:], zt[G_F][:], AF.Sigmoid)
                nc.scalar.activation(g_t["i"][:], zt[G_I][:], AF.Sigmoid)
                nc.scalar.activation(g_t["g"][:], zt[G_G][:], AF.Tanh)
                nc.scalar.activation(g_t["o"][:], zt[G_O][:], AF.Sigmoid)

            def emit_dve(t, st, c_prev, g_t):
                """Elementwise chain + tanh(c) + h for one stream."""
                m = tpool.tile([128, 2 * SB], BF16, tag="m", name=f"m{t}_{st}")
                fc = tpool.tile([128, 2 * SB], CDT, tag="fc", name=f"fc{t}_{st}")
                c_t = cpool.tile([128, 2 * SB], CDT, tag="c", name=f"c{t}_{st}")
                tc_t = gpool.tile([128, 2 * SB], BF16, tag="tc", name=f"tc{t}_{st}")
                h_t = hpool.tile([128, 2 * SB], BF16, tag="h", name=f"h{t}_{st}")

                if c_prev is not None:
                    nc.vector.tensor_mul(fc[:], g_t["f"][:], c_prev[:])
                nc.vector.tensor_mul(m[:], g_t["i"][:], g_t["g"][:])
                if c_prev is not None:
                    nc.vector.tensor_add(c_t[:], fc[:], m[:])
                else:
                    nc.vector.tensor_copy(c_t[:], m[:])
                nc.scalar.activation(tc_t[:], c_t[:], AF.Tanh)
                # h in unit-chunk halves: the next step's first u-pass only
                # needs cols 0:SB, so it can start one sem-hop earlier
                nc.vector.tensor_mul(h_t[:, 0:SB], g_t["o"][:, 0:SB], tc_t[:, 0:SB])
                nc.vector.tensor_mul(
                    h_t[:, SB : 2 * SB], g_t["o"][:, SB : 2 * SB], tc_t[:, SB : 2 * SB]
                )
                return h_t, c_t

            # ---- pred + output (per stream: 2 batch chunks of 128) ----
            def emit_pred_mm(s, st, h_t):
                pp = aux.tile([128, B], F32, tag="aux", name=f"pp{s}_{st}")
                for j in range(2):
                    dst = pp[:, F * j : F * (j + 1)]
                    nc.tensor.matmul(
                        dst, ones[0:1, 0:128], bdrow[:], start=True, stop=False
                    )
                    nc.tensor.matmul(
                        dst, h_t[:, 128 * j : 128 * (j + 1)], wdd2[:, 0:F],
                        start=False, stop=False,
                    )
                    nc.tensor.matmul(
                        dst, h_t[:, SB + 128 * j : SB + 128 * (j + 1)],
                        wdd2[:, F : 2 * F],
                        start=False, stop=True,
                    )
                return pp

            def emit_pred_out(s, st, pp):
                osb = opool.tile([128, 2 * F], F32, tag="ot", name=f"osb{s}_{st}")
                nc.vector.tensor_copy(osb[:], pp[:, 0 : 2 * F])
                nc.sync.dma_start(
                    yout_c[:, 2 * st : 2 * st + 2, F * s : F * (s + 1)],
                    osb[:].rearrange("p (c f) -> p c f", c=2),
                )

            # ---- prologue: stage x for the first steps ----
            dma_q = {}
            dma_q[0] = stage_x_dma(0)
            nc.sync.dma_start(ident[:], ident_d[:])
            nc.sync.dma_start(wb[:], wb_d[:])
            for t in range(1, min(XLOOK, T)):
                dma_q[t] = stage_x_dma(t)
            # big weight transfers after the first x tiles are in flight
            nc.sync.dma_start(u2[:], u2_d[:])
            nc.sync.dma_start(ones[:], ones_d[:])
            nc.sync.dma_start(ud2[:], ud2_d[:])
            nc.sync.dma_start(wdd2[:], wdd2_d[:])
            nc.sync.dma_start(bdec[:], bdec_d[:])
            nc.sync.dma_start(bdrow[:], bdrow_d[:])
            for t in range(min(XLOOK, T)):
                stage_x_transpose(t, dma_q.pop(t))
            if XLOOK < T:
                dma_q[XLOOK] = stage_x_dma(XLOOK)

            n_steps = T + (out_steps - 1)
            h_prev = [None] * NS
            c_prev = [None] * NS

            hs = {}
            for t in range(n_steps):
                warm = t < T
                x_t = xs[t % NXS] if warm else None
                uw = u2 if warm else ud2
                first = h_prev[0] is None

                # --- PE: recurrence blocks per stream ---
                zt = [dict() for _ in range(NS)]
                for st in range(NS):
                    for q in (G_F, G_I, G_G, G_O):
                        emit_gate(t, st, zt[st], x_t, q, uw, h_prev[st], first)

                # pred matmuls for the previous step's h (decode lags 1 step)
                if t >= T and (t - 1) in hs:
                    pps = [emit_pred_mm(t - T, st, hs[t - 1][st]) for st in range(NS)]

                # --- x staging for upcoming steps ---
                if t + 1 < n_steps:
                    if t + XLOOK < T and t + XLOOK in dma_q:
                        stage_x_transpose(t + XLOOK, dma_q.pop(t + XLOOK))
                    if t + XLOOK + 1 < T:
                        dma_q[t + XLOOK + 1] = stage_x_dma(t + XLOOK + 1)

                # --- Act/DVE tails, stream-interleaved ---
                g_ts = []
                for st in range(NS):
                    g_t = {
                        k: gpool.tile(
                            [128, 2 * SB], BF16, tag=f"g{k}", name=f"g{k}{t}_{st}"
                        )
                        for k in ("f", "i", "g", "o")
                    }
                    g_ts.append(g_t)
                    emit_acts(t, st, zt[st], c_prev[st], g_t)
                    h_prev[st], c_prev[st] = emit_dve(t, st, c_prev[st], g_t)

                # output DMA for lagged preds
                if t >= T and (t - 1) in hs:
                    for st in range(NS):
                        emit_pred_out(t - T, st, pps[st])
                    del hs[t - 1]
                if t >= T - 1:
                    hs[t] = list(h_prev)

            # epilogue: last pred
            for st in range(NS):
                pp = emit_pred_mm(out_steps - 1, st, hs[n_steps - 1][st])
                emit_pred_out(out_steps - 1, st, pp)

    nc.compile()
    return nc


_CACHE = {}


def _get_program(key):
    if key not in _CACHE:
        _CACHE[key] = build_program(*key)
    return _CACHE[key]


def _host_prep(W, Uk, b, Wd, bd):
    bf16 = mybir.dt.np(BF16)
    W64 = W.astype(np.float64)
    Ud = (Uk.astype(np.float64) + Wd.astype(np.float64) @ W64).astype(np.float32)
    bdec = (b.astype(np.float64) + bd.astype(np.float64) @ W64).astype(np.float32)
    wb = np.concatenate([W, b.reshape(1, -1)], axis=0)          # [65, 1024]
    u2 = np.concatenate([Uk[0:128], Uk[128:256]], axis=1)       # [128, 2048]
    ud2 = np.concatenate([Ud[0:128], Ud[128:256]], axis=1)
    wdd2 = np.concatenate([Wd[0:128], Wd[128:256]], axis=1)     # [128, 128]
    return {
        "wb": wb.astype(bf16),
        "u2": u2.astype(bf16),
        "ud2": ud2.astype(bf16),
        "wdd2": wdd2.astype(bf16),
        "bdec": bdec.reshape(1, -1).astype(bf16),
        "bdrow": bd.reshape(1, -1).astype(bf16),
        "ones": np.ones((1, SB), dtype=bf16),
        "ident": np.eye(128, dtype=np.float32),
    }


def kernel(inputs, W, U, b, Wd, bd, out_steps):
    inputs = np.asarray(inputs, dtype=np.float32)
    W = np.asarray(W, dtype=np.float32)
    U_ = np.asarray(U, dtype=np.float32)
    b_ = np.asarray(b, dtype=np.float32)
    Wd = np.asarray(Wd, dtype=np.float32)
    bd = np.asarray(bd, dtype=np.float32)
    out_steps = int(out_steps)

    B_full, T, _ = inputs.shape
    assert B_full % N_CORES == 0
    Bc = B_full // N_CORES

    nc = _get_program((Bc, T, out_steps))
    shared = _host_prep(W, U_, b_, Wd, bd)
    in_maps = [
        {"xin": np.ascontiguousarray(inputs[i * Bc : (i + 1) * Bc]), **shared}
        for i in range(N_CORES)
    ]
    res = bass_utils.run_bass_kernel_spmd(nc, in_maps, core_ids=list(range(N_CORES)))
    out = np.concatenate([res.results[i]["yout"] for i in range(N_CORES)], axis=0)
    return out
